# revision 2
# baseline (speedup 1.0000x reference)
"""Trainium2 Bass kernel for nn_EnhancedMoEModel (soft-clustered MoE inference).

Model (per row b of x[B,32], E=8 experts, H=64, H2=32):
    h1[e] = relu(x @ W1[e] + b1[e])            # [B,64] per expert
    h2[e] = relu(h1[e] @ W2[e] + b2[e])        # [B,32]
    eo[e] = sigmoid(h2[e] @ W3[e] + b3[e])     # [B,1]
    out[b] = sum_e probs[b,e] * eo[e][b]

Strategy: data-parallel over 8 NeuronCores (B=524288 -> 65536 rows/core).
All matmuls run weight-stationary in the transposed domain (units on
partitions, batch on the free axis, N=512 per block) with the expert dim
packed into partitions: expert pairs for layer 1 (2x64=128 outputs/matmul),
pair-block-diagonal weights for layer 2 ([128,64] -> 64-partition pair
tiles), and M-padded pair weights for layer 3 accumulating all 8 expert
logits into rows 0-7 of one PSUM bank. x is transposed on the host (layout
choice) with a ones-row appended so b1 rides inside the layer-1 matmul.
b2/b3 are applied by the per-partition bias operand of the PSUM->SBUF
evacuations, which also perform the relu/sigmoid (split between ScalarE and
VectorE - the throughput-critical path). The final combine transposes the
sigmoid outputs back to row-major via 4 tiny PE transposes and does
mult+grouped-reduce against natural-layout probs on VectorE.

Matmul operands use float32r (full fp32 storage, reduced-precision PE mode,
4x faster than fp32 matmul); set MM_DTYPE to float32 for exact-but-slow.
"""

import sys

sys.path.insert(0, "/opt/trn_rl_repo")

import numpy as np

from concourse import bacc, tile
from concourse.bass_utils import run_bass_kernel_spmd
import concourse.mybir as mybir

F32 = mybir.dt.float32
F32R = mybir.dt.float32r
AF = mybir.ActivationFunctionType
ALU = mybir.AluOpType

N_CORES = 8
B_FULL = 524288
D = 32
H = 64
H2 = 32
E = 8
B_SHARD = B_FULL // N_CORES  # 65536
BLK = 512                    # batch rows per block (PSUM bank free size)

MM_DTYPE = F32R              # float32r: 1 cyc/row PE; float32: 4 cyc/row


def build_nc(n_blocks):
    """Build the per-core Bass program for n_blocks 512-row blocks."""
    nc = bacc.Bacc("TRN2", target_bir_lowering=False, debug=False,
                   num_devices=N_CORES)
    nb = n_blocks
    rows = nb * BLK

    xT_d = nc.dram_tensor("xT", [D + 1, rows], MM_DTYPE, kind="ExternalInput")
    pr_d = nc.dram_tensor("probs", [rows, E], F32, kind="ExternalInput")
    w1_d = nc.dram_tensor("w1", [D + 1, 512], MM_DTYPE, kind="ExternalInput")
    w2_d = nc.dram_tensor("w2", [128, 256], MM_DTYPE, kind="ExternalInput")
    w3_d = nc.dram_tensor("w3", [64, 256], MM_DTYPE, kind="ExternalInput")
    b2_d = nc.dram_tensor("b2p", [64, 4], F32, kind="ExternalInput")
    b3_d = nc.dram_tensor("b3", [E, 1], F32, kind="ExternalInput")
    id_d = nc.dram_tensor("ident", [E, E], F32, kind="ExternalInput")
    out_d = nc.dram_tensor("out", [rows, 1], F32, kind="ExternalOutput")

    with tile.TileContext(nc) as tc:
        with (
            tc.tile_pool(name="const", bufs=1) as cpool,
            tc.tile_pool(name="xin", bufs=3) as xpool,
            tc.tile_pool(name="pin", bufs=3) as ppool,
            tc.tile_pool(name="h1sb", bufs=2) as h1pool,
            tc.tile_pool(name="h2sb", bufs=2) as h2pool,
            tc.tile_pool(name="tail", bufs=2) as tpool,
            tc.tile_pool(name="ps", bufs=3, space="PSUM") as psw,
            tc.tile_pool(name="ps_eo", bufs=1, space="PSUM") as pse,
            tc.tile_pool(name="ps_eT", bufs=1, space="PSUM") as psT,
        ):
            w1 = cpool.tile([D + 1, 512], MM_DTYPE, tag="w1")
            w2 = cpool.tile([128, 256], MM_DTYPE, tag="w2")
            w3 = cpool.tile([64, 256], MM_DTYPE, tag="w3")
            b2 = cpool.tile([64, 4], F32, tag="b2")
            b3 = cpool.tile([E, 1], F32, tag="b3")
            id8 = cpool.tile([E, E], F32, tag="id8")
            nc.sync.dma_start(out=w1[:], in_=w1_d[:])
            nc.sync.dma_start(out=w2[:], in_=w2_d[:])
            nc.sync.dma_start(out=w3[:], in_=w3_d[:])
            nc.sync.dma_start(out=b2[:], in_=b2_d[:])
            nc.sync.dma_start(out=b3[:], in_=b3_d[:])
            nc.sync.dma_start(out=id8[:], in_=id_d[:])

            for j in range(nb):
                b0 = j * BLK
                xt = xpool.tile([D + 1, BLK], MM_DTYPE, tag="xt")
                nc.sync.dma_start(out=xt[:], in_=xT_d[:, b0:b0 + BLK])
                pb = ppool.tile([128, 4 * E], F32, tag="pb")
                for c in range(4):
                    nc.sync.dma_start(
                        out=pb[:, c * E:(c + 1) * E],
                        in_=pr_d[b0 + c * 128:b0 + (c + 1) * 128, :])

                # ---- layer 1: x @ W1 (+b1 via ones-row), expert pairs ----
                h1A_ps = psw.tile([128, 1024], F32, tag="work")  # pairs 0,1
                h1D_ps = psw.tile([128, 1024], F32, tag="work")  # pairs 2,3
                for p in range(4):
                    dst = h1A_ps if p < 2 else h1D_ps
                    col = 512 * (p % 2)
                    nc.tensor.matmul(dst[:, col:col + 512],
                                     w1[:, 128 * p:128 * (p + 1)], xt[:],
                                     start=True, stop=True)
                h1A = h1pool.tile([128, 1024], MM_DTYPE, tag="h1A")
                h1D = h1pool.tile([128, 1024], MM_DTYPE, tag="h1D")
                nc.scalar.activation(h1A[:], h1A_ps[:], AF.Relu)
                nc.vector.tensor_scalar_max(h1D[:], h1D_ps[:], 0.0)

                # ---- layer 2: pair-block-diag W2 -> [64,512] pair halves ----
                h2T0_ps = psw.tile([64, 1024], F32, tag="work")  # pairs 0,1
                h2T1_ps = psw.tile([64, 1024], F32, tag="work")  # pairs 2,3
                nc.tensor.matmul(h2T0_ps[0:64, 0:512], w2[:, 0:64],
                                 h1A[:, 0:512], start=True, stop=True)
                nc.tensor.matmul(h2T0_ps[0:64, 512:1024], w2[:, 64:128],
                                 h1A[:, 512:1024], start=True, stop=True)
                nc.tensor.matmul(h2T1_ps[0:64, 0:512], w2[:, 128:192],
                                 h1D[:, 0:512], start=True, stop=True)
                nc.tensor.matmul(h2T1_ps[0:64, 512:1024], w2[:, 192:256],
                                 h1D[:, 512:1024], start=True, stop=True)
                h2T0 = h2pool.tile([64, 1024], MM_DTYPE, tag="h2T0")
                h2T1 = h2pool.tile([64, 1024], MM_DTYPE, tag="h2T1")
                nc.scalar.activation(h2T0[:, 0:512], h2T0_ps[:, 0:512],
                                     AF.Relu, bias=b2[:, 0:1])
                nc.scalar.activation(h2T0[:, 512:1024], h2T0_ps[:, 512:1024],
                                     AF.Relu, bias=b2[:, 1:2])
                nc.vector.tensor_scalar(h2T1[:, 0:512], h2T1_ps[:, 0:512],
                                        b2[:, 2:3], 0.0,
                                        op0=ALU.add, op1=ALU.max)
                nc.vector.tensor_scalar(h2T1[:, 512:1024],
                                        h2T1_ps[:, 512:1024], b2[:, 3:4], 0.0,
                                        op0=ALU.add, op1=ALU.max)

                # ---- layer 3: 4 accumulating pair matmuls -> eo rows 0-7 ----
                eo_ps = pse.tile([64, 512], F32, tag="eo")
                for p in range(4):
                    src = h2T0 if p < 2 else h2T1
                    col = 512 * (p % 2)
                    nc.tensor.matmul(eo_ps[:], w3[:, 64 * p:64 * (p + 1)],
                                     src[:, col:col + 512],
                                     start=(p == 0), stop=(p == 3))
                sig = tpool.tile([E, 512], F32, tag="sig")
                nc.scalar.activation(sig[:], eo_ps[0:E, :], AF.Sigmoid,
                                     bias=b3[:, 0:1])

                # ---- combine: transpose to row-major, dot with probs ----
                eT_ps = psT.tile([128, 4 * E], F32, tag="eT")
                for c in range(4):
                    nc.tensor.transpose(eT_ps[:, c * E:(c + 1) * E],
                                        sig[:, c * 128:(c + 1) * 128], id8[:])
                prod = tpool.tile([128, 4 * E], F32, tag="prod")
                nc.vector.tensor_tensor(prod[:], eT_ps[:], pb[:], op=ALU.mult)
                res = tpool.tile([128, 4], F32, tag="res")
                nc.vector.tensor_reduce(
                    res[:], prod[:].rearrange("p (c e) -> p c e", e=E),
                    axis=mybir.AxisListType.X, op=ALU.add)
                nc.sync.dma_start(
                    out=out_d[b0:b0 + BLK, :].rearrange("(c p) o -> p (c o)",
                                                        p=128),
                    in_=res[:])

    nc.compile()
    return nc


def prep_weights(W1, b1, W2, b2, W3, b3):
    """Host-side packing of the tiny expert weights."""
    f = np.float32
    w1 = np.zeros((D + 1, 512), f)
    for e in range(E):
        w1[:D, 64 * e:64 * (e + 1)] = W1[e]
        w1[D, 64 * e:64 * (e + 1)] = b1[e]
    w2 = np.zeros((128, 256), f)
    for p in range(4):
        for s in range(2):
            w2[64 * s:64 * (s + 1), 64 * p + 32 * s:64 * p + 32 * (s + 1)] = \
                W2[2 * p + s]
    # w3: per-pair [64,64] stationary, experts 2p/2p+1 in columns 2p/2p+1.
    w3 = np.zeros((64, 256), f)
    for p in range(4):
        for s in range(2):
            w3[32 * s:32 * (s + 1), 64 * p + 2 * p + s] = W3[2 * p + s, :, 0]
    b2p = np.zeros((64, 4), f)
    for p in range(4):
        for s in range(2):
            b2p[32 * s:32 * (s + 1), p] = b2[2 * p + s]
    b3c = np.ascontiguousarray(b3.reshape(E, 1), dtype=f)
    ident = np.eye(E, dtype=f)
    return {"w1": w1, "w2": w2, "w3": w3, "b2p": b2p, "b3": b3c,
            "ident": ident}


_NC_CACHE = {}


def _get_nc(n_blocks):
    if n_blocks not in _NC_CACHE:
        _NC_CACHE[n_blocks] = build_nc(n_blocks)
    return _NC_CACHE[n_blocks]


def kernel(x, soft_cluster_probs, W1, b1, W2, b2, W3, b3, _trace=False):
    x = np.asarray(x, np.float32)
    probs = np.asarray(soft_cluster_probs, np.float32)
    B = x.shape[0]
    assert B % N_CORES == 0
    shard = B // N_CORES
    assert shard % BLK == 0
    nb = shard // BLK

    wmap = prep_weights(np.asarray(W1, np.float32), np.asarray(b1, np.float32),
                        np.asarray(W2, np.float32), np.asarray(b2, np.float32),
                        np.asarray(W3, np.float32), np.asarray(b3, np.float32))

    xT = np.empty((D + 1, B), np.float32)
    xT[:D] = x.T
    xT[D] = 1.0

    in_maps = []
    for c in range(N_CORES):
        sl = slice(c * shard, (c + 1) * shard)
        m = {"xT": np.ascontiguousarray(xT[:, sl]),
             "probs": np.ascontiguousarray(probs[sl])}
        m.update(wmap)
        in_maps.append(m)

    nc = _get_nc(nb)
    kw = {}
    if _trace:
        kw = dict(trace=True)
    res = run_bass_kernel_spmd(nc, in_maps, core_ids=list(range(N_CORES)),
                               **kw)
    out = np.concatenate([res.results[c]["out"] for c in range(N_CORES)],
                         axis=0)
    kernel.last_exec_time_ns = res.exec_time_ns
    kernel.last_results = res
    return out


kernel.last_exec_time_ns = None
kernel.last_results = None
